# revision 23
# baseline (speedup 1.0000x reference)
"""CoOccurrenceLayer Trainium2 kernel (8 NeuronCores, data-parallel over batch).

Algorithm: out[p] = sum_{dq in 5x5} filt[dq] * co[idx[p], idx[p+dq]] * x[p+dq]
where idx is a 16-bin quantization of exp(x) normalized by global min/max.

v2 structure (chunk-major; vs v1 drops the 15-op threshold chain, the idx
replication pass, the replicated-idx transpose, the PE select-reduce and the
8-partition output strips):
  * Binning on the ACT engine: e = exp(x); t = |a*e + b|; idx = rne(t-0.5)
    cast to int16 (3 ops; device exp is bit-identical to jax-on-device exp).
  * 2D-blocked layout: padded image [520, 528] in 8x16 blocks; partition =
    (r%8)*16 + c%16, free = block index. All 25 conv taps live inside a 2x2
    neighborhood of input blocks.
  * Per (chunk of 512 out-blocks, bin): scatter V = x*(idx==j) over the
    chunk's 17 block-rows; 5x5 conv on the TensorEngine as 4 PSUM-accumulated
    matmuls against static filter-structured [128,128] weights; ACT evacuates
    psum into a j-interleaved C chunk [128, (g, j, blk8)].
  * Mix D_i = sum_j co[i,j] C_j on the TensorEngine: DMA-xbar transpose of
    the C chunk to channel-major, 16 matmuls vs the co-structured weight,
    evacuate, transpose back to pixel-major.
  * Select out[p] = D_{idx[p]}[p] via copy_predicated with int16 masks
    (idxo == i), writing the final f16 output tile directly.
"""

import sys

sys.path.insert(0, "/opt/trn_rl_repo")

import numpy as np

import concourse.bacc as bacc
import concourse.mybir as mybir
import concourse.tile as tile
from concourse import bass_utils
from concourse.ap import AP

# ---------------------------------------------------------------- constants
B, HH, WW = 64, 512, 512
NCORES = 8
BPC = B // NCORES
NQ = 16
EPS = 1e-5

AV, BV = 65, 33            # V-grid blocks of 8x16 (padded image 520x528)
NV = AV * BV               # 2145
AO, BO = 64, 32            # out grid
NO = AO * BO               # 2048 out-blocks
NCH = 4                    # chunks of 512 out-blocks (16 block-rows each)
CW = 512                   # out-blocks per chunk
VW = 17 * BV               # V-blocks touched by one chunk (17 block-rows)

F16 = mybir.dt.float16
F32 = mybir.dt.float32
I16 = mybir.dt.int16
ALU = mybir.AluOpType
ACT = mybir.ActivationFunctionType


# ------------------------------------------------------- static PE weights
def build_weights(co, filt):
    """wt [128, 5*128] f16: 4 conv lhsT (2x2 block nbhd) + 1 mix lhsT."""
    W = np.zeros((5, 128, 128), np.float32)
    for da in range(2):
        for db in range(2):
            v = da * 2 + db
            for kr in range(8):
                for kc in range(16):
                    for mr in range(8):
                        for mc in range(16):
                            dr = 8 * da - 2 + kr - mr
                            dc = 16 * db - 2 + kc - mc
                            if -2 <= dr <= 2 and -2 <= dc <= 2:
                                W[v, kr * 16 + kc, mr * 16 + mc] = filt[dr + 2, dc + 2]
    # mix: k = (j, blk8), m = (i, blk8'): co[i, j] when blk8 == blk8'
    for j in range(NQ):
        for blk8 in range(8):
            for i in range(NQ):
                W[4, j * 8 + blk8, i * 8 + blk8] = co[i, j]
    return np.ascontiguousarray(
        W.astype(np.float16).transpose(1, 0, 2).reshape(128, 5 * 128)
    )


# ------------------------------------------------------- device program
def build_program(a_mul, b_add):
    nc = bacc.Bacc("TRN2", target_bir_lowering=False, debug=False)
    x_d = nc.dram_tensor("x", [BPC, 128, NV], F32, kind="ExternalInput").ap()
    w_d = nc.dram_tensor("wt", [128, 5 * 128], F16, kind="ExternalInput").ap()
    o_d = nc.dram_tensor("out", [BPC, 128, NO], F16, kind="ExternalOutput").ap()

    with tile.TileContext(nc) as tc:
        with (
            tc.tile_pool(name="wp", bufs=1) as p_w,
            tc.tile_pool(name="xs", bufs=2) as p_xs,
            tc.tile_pool(name="scr", bufs=2) as p_scr,
            tc.tile_pool(name="bins", bufs=2) as p_bin,
            tc.tile_pool(name="msk", bufs=2) as p_m,
            tc.tile_pool(name="vpl", bufs=3) as p_v,
            tc.tile_pool(name="cc", bufs=1) as p_c,
            tc.tile_pool(name="ct", bufs=2) as p_ct,
            tc.tile_pool(name="dts", bufs=2) as p_dts,
            tc.tile_pool(name="dn", bufs=2) as p_dn,
            tc.tile_pool(name="accp", bufs=2) as p_acc,
            tc.tile_pool(name="cps", bufs=4, space="PSUM") as p_cps,
            tc.tile_pool(name="mps", bufs=2, space="PSUM") as p_mps,
        ):
            wt = p_w.tile([128, 5 * 128], F16)
            nc.sync.dma_start(wt[:], w_d[:])
            bias = p_w.tile([128, 1], F32)
            nc.vector.memset(bias[:], float(b_add))

            for img in range(BPC):
                xs = p_xs.tile([128, NV], F32, tag="xs")
                nc.sync.dma_start(xs[:], x_d[img])

                # --- binning: idx = rne(|a*exp(x) + b| - 0.5) as int16 ---
                ee = p_scr.tile([128, NV], F32, tag="scr", name="ee")
                nc.scalar.activation(ee[:], xs[:], ACT.Exp)
                tt = p_scr.tile([128, NV], F32, tag="scr", name="tt")
                nc.scalar.activation(
                    tt[:], ee[:], ACT.Abs, bias=bias[:], scale=float(a_mul)
                )
                idxi = p_bin.tile([128, NV], I16, tag="idxi", bufs=1)
                nc.vector.tensor_scalar(idxi[:], tt[:], 0.5, None, ALU.subtract)
                idx = p_bin.tile([128, NV], F16, tag="idx")
                nc.vector.tensor_copy(idx[:], idxi[:])
                x16 = p_bin.tile([128, NV], F16, tag="x16")
                nc.scalar.copy(x16[:], xs[:])

                # --- idx in out-grid layout (phase shift +2,+2) ---
                idxo = p_bin.tile([128, NO], F16, tag="idxo")
                src_t = idx[:]
                dst_t = idxo[:]
                for rcase in range(2):
                    for ccase in range(2):
                        nmr = 6 if rcase == 0 else 2
                        nmc = 14 if ccase == 0 else 2
                        mr0 = 0 if rcase == 0 else 6
                        mc0 = 0 if ccase == 0 else 14
                        soff = (1 if rcase else 0) * BV + (1 if ccase else 0)
                        for mr in range(mr0, mr0 + nmr):
                            spart = ((mr + 2) % 8) * 16 + ((mc0 + 2) % 16)
                            dpart = mr * 16 + mc0
                            src = AP(
                                src_t.tensor,
                                src_t.offset + spart * NV + soff,
                                [[NV, nmc], [BV, AO], [1, BO]],
                            )
                            dst = AP(
                                dst_t.tensor,
                                dst_t.offset + dpart * NO,
                                [[NO, nmc], [BO, AO], [1, BO]],
                            )
                            nc.sync.dma_start(dst, src)

                acc = p_acc.tile([128, NO], F16, tag="acc")

                for c in range(NCH):
                    v0 = c * 16 * BV  # first V-block column of this chunk
                    osl = slice(c * CW, (c + 1) * CW)

                    # --- conv per bin into j-interleaved C chunk ---
                    # waves of 4 bins, weight-major inside a wave (4 ldweights
                    # per wave); psum paired [128, 1024] so each ACT evac
                    # moves two bins at once
                    cc = p_c.tile([128, 64 * 128], F16, tag="cc")
                    for w0 in range(0, NQ, 6):
                        wave = range(w0, min(w0 + 6, NQ))
                        vjs = {}
                        pss = {}
                        for j in wave:
                            vj = p_v.tile([128, VW], F16, tag=f"vj{j % 6}",
                                          name="vj", bufs=2)
                            nc.vector.scalar_tensor_tensor(
                                vj[:], idx[:, v0 : v0 + VW], float(j),
                                x16[:, v0 : v0 + VW], ALU.is_equal, ALU.mult,
                            )
                            vjs[j] = vj
                            pss[j] = p_cps.tile([128, CW], F32, tag=f"ps{j % 6}",
                                                name="ps", bufs=1)
                        for v in range(4):
                            da, db = v >> 1, v & 1
                            for j in wave:
                                vt = vjs[j][:]
                                rhs = AP(
                                    vt.tensor,
                                    vt.offset + da * BV + db,
                                    [[VW, 128], [BV, 16], [1, BO]],
                                )
                                nc.tensor.matmul(
                                    pss[j][:],
                                    wt[:, v * 128 : (v + 1) * 128],
                                    rhs,
                                    start=(v == 0),
                                    stop=(v == 3),
                                )
                        for j in wave:
                            # evac psum -> C[(g, j, blk8)] (strided dst)
                            cv = cc[:]
                            dst = AP(
                                cv.tensor,
                                cv.offset + j * 8,
                                [cv.ap[0], [128, 64], [1, 8]],
                            )
                            if j in (5, 11, 15):
                                nc.vector.tensor_copy(dst, pss[j][:])
                            else:
                                nc.scalar.copy(dst, pss[j][:])

                    # --- transpose to channel-major ---
                    ct = p_ct.tile([128, 64, 128], F16, tag="ct")
                    for h in range(2):
                        nc.sync.dma_start_transpose(
                            ct[:, h * 32 : (h + 1) * 32, :],
                            cc[:, h * 32 * 128 : (h + 1) * 32 * 128],
                        )

                    # --- mix on PE: D[(i,blk8), n] = sum_j co[i,j] C[(j,blk8), n] ---
                    dts = p_dts.tile([128, 64 * 128], F16, tag="dts")
                    ctf = ct[:].rearrange("p a b -> p (a b)")
                    for m in range(16):
                        psd = p_mps.tile([128, CW], F32, tag="psd")
                        nc.tensor.matmul(
                            psd[:],
                            wt[:, 4 * 128 : 5 * 128],
                            ctf[:, m * CW : (m + 1) * CW],
                            start=True,
                            stop=True,
                        )
                        if m % 8 == 0:
                            nc.vector.tensor_copy(
                                dts[:, m * CW : (m + 1) * CW], psd[:]
                            )
                        else:
                            nc.scalar.copy(dts[:, m * CW : (m + 1) * CW], psd[:])

                    # --- transpose back to pixel-major ---
                    dn = p_dn.tile([128, 64, 128], F16, tag="dn")
                    for h in range(2):
                        nc.scalar.dma_start_transpose(
                            dn[:, h * 32 : (h + 1) * 32, :],
                            dts[:, h * 32 * 128 : (h + 1) * 32 * 128],
                        )

                    # --- select: acc = D_i where idxo == i ---
                    dv = dn[:].rearrange("p a b -> p (a b)")
                    for i in range(NQ):
                        dsl = AP(
                            dv.tensor,
                            dv.offset + i * 8,
                            [dv.ap[0], [128, 64], [1, 8]],
                        )
                        if i == 0:
                            nc.vector.tensor_copy(acc[:, osl], dsl)
                        else:
                            mk = p_m.tile([128, CW], I16, tag="mk", name="mk")
                            nc.vector.tensor_scalar(
                                mk[:], idxo[:, osl], float(i), None, ALU.is_equal
                            )
                            nc.vector.copy_predicated(acc[:, osl], mk[:], dsl)

                nc.sync.dma_start(o_d[img], acc[:])

    nc.compile()
    return nc


# ------------------------------------------------------- host packing
def pack_inputs(x):
    imgs = x[:, 0]
    xpad = np.pad(imgs, ((0, 0), (2, 6), (2, 14)))      # [64, 520, 528]
    xb = (
        xpad.reshape(B, AV, 8, BV, 16)
        .transpose(0, 2, 4, 1, 3)
        .reshape(B, 128, NV)
    )
    return np.ascontiguousarray(xb)


def unpack_outputs(res_list):
    out = np.empty((B, 1, HH, WW), np.float32)
    for c in range(NCORES):
        ob = res_list[c]["out"].astype(np.float32)       # [BPC, 128, NO]
        o = ob.reshape(BPC, 8, 16, AO, BO)               # mr, mc, a, b
        o = o.transpose(0, 3, 1, 4, 2)                   # img, a, mr, b, mc
        out[c * BPC : (c + 1) * BPC, 0] = o.reshape(BPC, HH, WW)
    return out


def bin_constants(x):
    """t = |a*exp(x) + b|; idx = floor(t) matches the reference chain
    floor(|16*(e-m)/M - eps|) to within ~1ulp of t."""
    import jax.numpy as jnp

    xmin = np.float32(x.min())
    xmax = np.float32(x.max())
    m = np.float64(np.asarray(jnp.exp(xmin), dtype=np.float32))
    M = np.float64(np.asarray(jnp.exp(xmax), dtype=np.float32))
    a = np.float32(16.0 / M)
    b = np.float32(-(16.0 * m / M + EPS))
    return a, b


def kernel(x, co_matrix, spatial_filter):
    x = np.asarray(x, np.float32)
    co = np.asarray(co_matrix, np.float32)
    filt = np.asarray(spatial_filter, np.float32)

    a_mul, b_add = bin_constants(x)
    xb = pack_inputs(x)
    wts = build_weights(co, filt)

    nc = build_program(a_mul, b_add)
    in_maps = [
        {"x": xb[c * BPC : (c + 1) * BPC], "wt": wts}
        for c in range(NCORES)
    ]
    res = bass_utils.run_bass_kernel_spmd(nc, in_maps, core_ids=list(range(NCORES)))
    return unpack_outputs(res.results)


# revision 26
# speedup vs baseline: 1.0210x; 1.0210x over previous
"""CoOccurrenceLayer Trainium2 kernel (8 NeuronCores, data-parallel over batch).

Algorithm: out[p] = sum_{dq in 5x5} filt[dq] * co[idx[p], idx[p+dq]] * x[p+dq]
where idx is a 16-bin quantization of exp(x) normalized by global min/max.

v2 structure (chunk-major; vs v1 drops the 15-op threshold chain, the idx
replication pass, the replicated-idx transpose, the PE select-reduce and the
8-partition output strips):
  * Binning on the ACT engine: e = exp(x); t = |a*e + b|; idx = rne(t-0.5)
    cast to int16 (3 ops; device exp is bit-identical to jax-on-device exp).
  * 2D-blocked layout: padded image [520, 528] in 8x16 blocks; partition =
    (r%8)*16 + c%16, free = block index. All 25 conv taps live inside a 2x2
    neighborhood of input blocks.
  * Per (chunk of 512 out-blocks, bin): scatter V = x*(idx==j) over the
    chunk's 17 block-rows; 5x5 conv on the TensorEngine as 4 PSUM-accumulated
    matmuls against static filter-structured [128,128] weights; ACT evacuates
    psum into a j-interleaved C chunk [128, (g, j, blk8)].
  * Mix D_i = sum_j co[i,j] C_j on the TensorEngine: DMA-xbar transpose of
    the C chunk to channel-major, 16 matmuls vs the co-structured weight,
    evacuate, transpose back to pixel-major.
  * Select out[p] = D_{idx[p]}[p] via copy_predicated with int16 masks
    (idxo == i), writing the final f16 output tile directly.
"""

import sys

sys.path.insert(0, "/opt/trn_rl_repo")

import numpy as np

import concourse.bacc as bacc
import concourse.mybir as mybir
import concourse.tile as tile
from concourse import bass_utils
from concourse.ap import AP

# ---------------------------------------------------------------- constants
B, HH, WW = 64, 512, 512
NCORES = 8
BPC = B // NCORES
NQ = 16
EPS = 1e-5

AV, BV = 65, 33            # V-grid blocks of 8x16 (padded image 520x528)
NV = AV * BV               # 2145
AO, BO = 64, 32            # out grid
NO = AO * BO               # 2048 out-blocks
NCH = 4                    # chunks of 512 out-blocks (16 block-rows each)
CW = 512                   # out-blocks per chunk
VW = 17 * BV               # V-blocks touched by one chunk (17 block-rows)

F16 = mybir.dt.float16
F32 = mybir.dt.float32
I16 = mybir.dt.int16
ALU = mybir.AluOpType
ACT = mybir.ActivationFunctionType


# ------------------------------------------------------- static PE weights
def build_weights(co, filt):
    """wt [128, 5*128] f16: 4 conv lhsT (2x2 block nbhd) + 1 mix lhsT."""
    W = np.zeros((5, 128, 128), np.float32)
    for da in range(2):
        for db in range(2):
            v = da * 2 + db
            for kr in range(8):
                for kc in range(16):
                    for mr in range(8):
                        for mc in range(16):
                            dr = 8 * da - 2 + kr - mr
                            dc = 16 * db - 2 + kc - mc
                            if -2 <= dr <= 2 and -2 <= dc <= 2:
                                W[v, kr * 16 + kc, mr * 16 + mc] = filt[dr + 2, dc + 2]
    # mix: k = (j, blk8), m = (i, blk8'): co[i, j] when blk8 == blk8'
    for j in range(NQ):
        for blk8 in range(8):
            for i in range(NQ):
                W[4, j * 8 + blk8, i * 8 + blk8] = co[i, j]
    return np.ascontiguousarray(
        W.astype(np.float16).transpose(1, 0, 2).reshape(128, 5 * 128)
    )


# ------------------------------------------------------- device program
def build_program(a_mul, b_add):
    nc = bacc.Bacc("TRN2", target_bir_lowering=False, debug=False)
    x_d = nc.dram_tensor("x", [BPC, 128, NV], F32, kind="ExternalInput").ap()
    w_d = nc.dram_tensor("wt", [128, 5 * 128], F16, kind="ExternalInput").ap()
    o_d = nc.dram_tensor("out", [BPC, 128, NO], F16, kind="ExternalOutput").ap()

    with tile.TileContext(nc) as tc:
        with (
            tc.tile_pool(name="wp", bufs=1) as p_w,
            tc.tile_pool(name="xs", bufs=2) as p_xs,
            tc.tile_pool(name="scr", bufs=2) as p_scr,
            tc.tile_pool(name="bins", bufs=2) as p_bin,
            tc.tile_pool(name="msk", bufs=2) as p_m,
            tc.tile_pool(name="vpl", bufs=3) as p_v,
            tc.tile_pool(name="cc", bufs=1) as p_c,
            tc.tile_pool(name="ct", bufs=2) as p_ct,
            tc.tile_pool(name="dts", bufs=2) as p_dts,
            tc.tile_pool(name="dn", bufs=2) as p_dn,
            tc.tile_pool(name="accp", bufs=2) as p_acc,
            tc.tile_pool(name="cps", bufs=4, space="PSUM") as p_cps,
            tc.tile_pool(name="mps", bufs=2, space="PSUM") as p_mps,
        ):
            wt = p_w.tile([128, 5 * 128], F16)
            nc.sync.dma_start(wt[:], w_d[:])
            bias = p_w.tile([128, 1], F32)
            nc.vector.memset(bias[:], float(b_add))

            for img in range(BPC):
                xs = p_xs.tile([128, NV], F32, tag="xs")
                nc.sync.dma_start(xs[:], x_d[img])

                # --- binning: idx = rne(|a*exp(x) + b| - 0.5) as int16 ---
                ee = p_scr.tile([128, NV], F32, tag="scr", name="ee")
                nc.scalar.activation(ee[:], xs[:], ACT.Exp)
                tt = p_scr.tile([128, NV], F32, tag="scr", name="tt")
                nc.scalar.activation(
                    tt[:], ee[:], ACT.Abs, bias=bias[:], scale=float(a_mul)
                )
                idxi = p_bin.tile([128, NV], I16, tag="idxi", bufs=1)
                nc.vector.tensor_scalar(idxi[:], tt[:], 0.5, None, ALU.subtract)
                idx = p_bin.tile([128, NV], F16, tag="idx")
                nc.vector.tensor_copy(idx[:], idxi[:])
                x16 = p_bin.tile([128, NV], F16, tag="x16")
                nc.scalar.copy(x16[:], xs[:])

                # --- idx in out-grid layout (phase shift +2,+2) ---
                idxo = p_bin.tile([128, NO], F16, tag="idxo")
                src_t = idx[:]
                dst_t = idxo[:]
                for rcase in range(2):
                    for ccase in range(2):
                        nmr = 6 if rcase == 0 else 2
                        nmc = 14 if ccase == 0 else 2
                        mr0 = 0 if rcase == 0 else 6
                        mc0 = 0 if ccase == 0 else 14
                        soff = (1 if rcase else 0) * BV + (1 if ccase else 0)
                        for mr in range(mr0, mr0 + nmr):
                            spart = ((mr + 2) % 8) * 16 + ((mc0 + 2) % 16)
                            dpart = mr * 16 + mc0
                            src = AP(
                                src_t.tensor,
                                src_t.offset + spart * NV + soff,
                                [[NV, nmc], [BV, AO], [1, BO]],
                            )
                            dst = AP(
                                dst_t.tensor,
                                dst_t.offset + dpart * NO,
                                [[NO, nmc], [BO, AO], [1, BO]],
                            )
                            nc.sync.dma_start(dst, src)

                acc = p_acc.tile([128, NO], F16, tag="acc")

                for c in range(NCH):
                    v0 = c * 16 * BV  # first V-block column of this chunk
                    osl = slice(c * CW, (c + 1) * CW)

                    # --- conv per bin into j-interleaved C chunk ---
                    # waves of 4 bins, weight-major inside a wave (4 ldweights
                    # per wave); psum paired [128, 1024] so each ACT evac
                    # moves two bins at once
                    cc = p_c.tile([128, 64 * 128], F16, tag="cc")
                    for w0 in range(0, NQ, 6):
                        wave = range(w0, min(w0 + 6, NQ))
                        vjs = {}
                        pss = {}
                        for j in wave:
                            mv = p_v.tile([128, VW], F16, tag=f"mv{j % 6}",
                                          name="mv", bufs=1)
                            nc.vector.tensor_scalar(
                                mv[:], idx[:, v0 : v0 + VW], float(j), None,
                                ALU.is_equal,
                            )
                            vj = p_v.tile([128, VW], F16, tag=f"vj{j % 6}",
                                          name="vj", bufs=2)
                            nc.vector.tensor_tensor(
                                vj[:], mv[:], x16[:, v0 : v0 + VW], ALU.mult
                            )
                            vjs[j] = vj
                            pss[j] = p_cps.tile([128, CW], F32, tag=f"ps{j % 6}",
                                                name="ps", bufs=1)
                        for v in range(4):
                            da, db = v >> 1, v & 1
                            for j in wave:
                                vt = vjs[j][:]
                                rhs = AP(
                                    vt.tensor,
                                    vt.offset + da * BV + db,
                                    [[VW, 128], [BV, 16], [1, BO]],
                                )
                                nc.tensor.matmul(
                                    pss[j][:],
                                    wt[:, v * 128 : (v + 1) * 128],
                                    rhs,
                                    start=(v == 0),
                                    stop=(v == 3),
                                )
                        for j in wave:
                            # evac psum -> C[(g, j, blk8)] (strided dst)
                            cv = cc[:]
                            dst = AP(
                                cv.tensor,
                                cv.offset + j * 8,
                                [cv.ap[0], [128, 64], [1, 8]],
                            )
                            if j in (5, 11, 15):
                                nc.vector.tensor_copy(dst, pss[j][:])
                            else:
                                nc.scalar.copy(dst, pss[j][:])

                    # --- transpose to channel-major ---
                    ct = p_ct.tile([128, 64, 128], F16, tag="ct")
                    nc.sync.dma_start_transpose(ct[:], cc[:])

                    # --- mix on PE: D[(i,blk8), n] = sum_j co[i,j] C[(j,blk8), n] ---
                    dts = p_dts.tile([128, 64 * 128], F16, tag="dts")
                    ctf = ct[:].rearrange("p a b -> p (a b)")
                    for m in range(16):
                        psd = p_mps.tile([128, CW], F32, tag="psd")
                        nc.tensor.matmul(
                            psd[:],
                            wt[:, 4 * 128 : 5 * 128],
                            ctf[:, m * CW : (m + 1) * CW],
                            start=True,
                            stop=True,
                        )
                        if m % 8 == 0:
                            nc.vector.tensor_copy(
                                dts[:, m * CW : (m + 1) * CW], psd[:]
                            )
                        else:
                            nc.scalar.copy(dts[:, m * CW : (m + 1) * CW], psd[:])

                    # --- transpose back to pixel-major ---
                    dn = p_dn.tile([128, 64, 128], F16, tag="dn")
                    nc.scalar.dma_start_transpose(dn[:], dts[:])

                    # --- select: acc = D_i where idxo == i ---
                    dv = dn[:].rearrange("p a b -> p (a b)")
                    for i in range(NQ):
                        dsl = AP(
                            dv.tensor,
                            dv.offset + i * 8,
                            [dv.ap[0], [128, 64], [1, 8]],
                        )
                        if i == 0:
                            nc.vector.tensor_copy(acc[:, osl], dsl)
                        else:
                            mk = p_m.tile([128, CW], I16, tag="mk", name="mk")
                            nc.vector.tensor_scalar(
                                mk[:], idxo[:, osl], float(i), None, ALU.is_equal
                            )
                            nc.vector.copy_predicated(acc[:, osl], mk[:], dsl)

                nc.sync.dma_start(o_d[img], acc[:])

    nc.compile()
    return nc


# ------------------------------------------------------- host packing
def pack_inputs(x):
    imgs = x[:, 0]
    xpad = np.pad(imgs, ((0, 0), (2, 6), (2, 14)))      # [64, 520, 528]
    xb = (
        xpad.reshape(B, AV, 8, BV, 16)
        .transpose(0, 2, 4, 1, 3)
        .reshape(B, 128, NV)
    )
    return np.ascontiguousarray(xb)


def unpack_outputs(res_list):
    out = np.empty((B, 1, HH, WW), np.float32)
    for c in range(NCORES):
        ob = res_list[c]["out"].astype(np.float32)       # [BPC, 128, NO]
        o = ob.reshape(BPC, 8, 16, AO, BO)               # mr, mc, a, b
        o = o.transpose(0, 3, 1, 4, 2)                   # img, a, mr, b, mc
        out[c * BPC : (c + 1) * BPC, 0] = o.reshape(BPC, HH, WW)
    return out


def bin_constants(x):
    """t = |a*exp(x) + b|; idx = floor(t) matches the reference chain
    floor(|16*(e-m)/M - eps|) to within ~1ulp of t."""
    import jax.numpy as jnp

    xmin = np.float32(x.min())
    xmax = np.float32(x.max())
    m = np.float64(np.asarray(jnp.exp(xmin), dtype=np.float32))
    M = np.float64(np.asarray(jnp.exp(xmax), dtype=np.float32))
    a = np.float32(16.0 / M)
    b = np.float32(-(16.0 * m / M + EPS))
    return a, b


def kernel(x, co_matrix, spatial_filter):
    x = np.asarray(x, np.float32)
    co = np.asarray(co_matrix, np.float32)
    filt = np.asarray(spatial_filter, np.float32)

    a_mul, b_add = bin_constants(x)
    xb = pack_inputs(x)
    wts = build_weights(co, filt)

    nc = build_program(a_mul, b_add)
    in_maps = [
        {"x": xb[c * BPC : (c + 1) * BPC], "wt": wts}
        for c in range(NCORES)
    ]
    res = bass_utils.run_bass_kernel_spmd(nc, in_maps, core_ids=list(range(NCORES)))
    return unpack_outputs(res.results)
